# revision 1
# baseline (speedup 1.0000x reference)
# Trainium2 Bass kernel for masked (key-padding) attention layer.
#
#   q,k,v = x@Wq, x@Wk, x@Wv ; score = q@k^T/sqrt(T) masked over keys;
#   out = softmax(score)@v @ Wo
#
# Sharding: data-parallel over batch, B=8 -> one batch element per NeuronCore,
# weights broadcast on-device via AllGather (host ships one row-shard per core).
#
# The dominant cost in this deployment is the host<->device tunnel (~60-90
# MB/s) plus per-call jax dispatch, NOT device execution (~150us). So the
# kernel is organized to minimize bytes moved per call and to reuse one cached
# jitted executable:
#   - x ships once, bf16, in its natural [T, D] layout (no host transpose);
#     the device transposes it with the PE array (matmul-with-identity).
#   - the folded weights (A = Wq@Wk^T, Avo = Wv@Wo, computed on host: 2 tiny
#     512^3 GEMMs) ship row-SHARDED (1/8th per core) and are AllGathered
#     on-device over NeuronLink instead of being replicated over the tunnel.
#   - all per-call inputs (x, weight shards, key bias) ship as ONE bf16 blob
#     per core (extra arrays each cost ~50ms of per-array transfer overhead).
#   - the output is written in natural [T, D] layout (stage C emits the
#     [t, o] orientation directly), quantized to int8 with a per-row scale
#     (absmax/127, rel err ~7e-3, well under the 2e-2 gate) to halve the
#     slower downstream leg; the host dequantizes in one fused pass.
#   - the donated "zero" output operands the bass custom-call needs are
#     device-resident: first call materializes them with a tiny on-device jit;
#     later calls recycle the previous call's (already copied) output buffer.
#
# Per-core algorithm (everything keyed off the pre-folded weights), in PE
# order transposes -> A1 -> B -> A2 -> C. The A.T gather lands first so A1
# starts ~28us in; the Avo gather and the denominator all-reduce hide under
# B and C respectively (TimelineSim: 187.6us/core):
#   xT = transpose(x)                       (PE transpose, 64 128x128 blocks)
#   u[x,j]  = sum_x' A[x,x'] xT[x',j]       (A1, 64 MMs)
#   sT[j,t] = sum_x u[x,j] xT[x,t]          (B, 256 MMs)
#   eT = exp(sT/sqrt(T) + kbias)            (ScalarE, PSUM->SBUF bf16)
#   v2[j,o] = sum_x xT[x,j] Avo[x,o]        (A2, 64 MMs)
#   den[t] = sum_j eT[j,t]: DVE row-accumulate + gpsimd partition
#     all-reduce, shipped RAW as 16 extra output rows (host folds 1/den
#     into the dequant scale, so stage C never waits on it)
#   out[t,o] = sum_j eT[j,t] v2[j,o]        (C, 256 MMs, natural [t, o]
#   orientation: lhsT=eT chunk, rhs=v2 -> PSUM -> per-row int8 quant -> DMA)
import math

import numpy as np
import ml_dtypes

B = 8
T = 2048
D = 512
P = 128
KC = D // P       # 4 contraction chunks of 128
QB = 512          # free-dim chunk (one PSUM bank of f32)
NQ = T // QB      # 4 query chunks
NT = T // P       # 16 tiles of 128
WSH = D // B      # 64-row weight shard per core
SCALE = 1.0 / math.sqrt(float(T))
PAD_BIAS = -30000.0

_BF16 = ml_dtypes.bfloat16

_ctx: dict = {}


def _build():
    """Build + compile the single-core SPMD program (mask-independent)."""
    import concourse.bass as bass
    import concourse.bass_isa as bass_isa
    import concourse.mybir as mybir
    import concourse.tile as tile
    from concourse import bacc

    dt = mybir.dt
    f32, bf16 = dt.float32, dt.bfloat16

    nc = bacc.Bacc(
        "TRN2",
        target_bir_lowering=False,
        debug=False,
        enable_asserts=False,
        num_devices=B,
    )

    i8 = dt.int8

    # one input blob per core: rows 0..T-1 = x [T, D]; rows T..T+63 = AT
    # shard; rows T+64..T+127 = Avo shard; rows T+128..T+131 = key bias
    # (bf16, [128, 16] flattened row-major).
    blob_d = nc.dram_tensor("blob", [T + 2 * WSH + 4, D], bf16,
                            kind="ExternalInput")
    ident_d = nc.dram_tensor("ident", [P, P], bf16, kind="ExternalInput")
    # out rows 0..T-1: per-row int8 quantized output of the UNNORMALIZED
    # attention sum; rows T..T+15: per-row f32 absmax/127 scales, bit-packed
    # (scl[p, n] = scale of row t = n*128+p); rows T+16..T+31: the f32 softmax
    # denominators in linear t order. The host folds den into the scale, so
    # stage C never waits on the denominator reduction.
    out_d = nc.dram_tensor("out", [T + 2 * NT, D], i8, kind="ExternalOutput")

    Exp = mybir.ActivationFunctionType.Exp

    with tile.TileContext(nc) as tc:
        with (
            tc.tile_pool(name="const", bufs=1) as cpool,
            tc.tile_pool(name="big", bufs=1) as bpool,
            tc.tile_pool(name="psum", bufs=6, space="PSUM") as psum,
            tc.tile_pool(name="psumt", bufs=2, space="PSUM") as psumt,
            tc.tile_pool(name="outs", bufs=4) as opool,
            tc.tile_pool(name="dram", bufs=1, space="DRAM") as dram,
        ):
            # ---- persistent SBUF tensors ----
            xrow = bpool.tile([P, NT, D], bf16, tag="xrow")
            xT = bpool.tile([P, KC, T], bf16, tag="xT")
            AT = cpool.tile([P, KC, D], bf16, tag="AT")
            Avo = cpool.tile([P, KC, D], bf16, tag="Avo")
            kbias16 = cpool.tile([P, NT], bf16, tag="kbias16")
            kbias = cpool.tile([P, NT], f32, tag="kbias")
            ident = cpool.tile([P, P], bf16, tag="ident")
            u = bpool.tile([P, KC, T], bf16, tag="u")
            v2 = bpool.tile([P, NT, D], bf16, tag="v2")
            eT = bpool.tile([P, NT, T], bf16, tag="eT")
            dacc = bpool.tile([P, T], f32, tag="dacc")
            rbc = bpool.tile([P, T], f32, tag="rbc")

            # ---- weight shards: DRAM->DRAM bounce, AllGather over
            # NeuronLink, then load gathered [D, D] into SBUF. Issued first
            # so the comm overlaps the x load + transpose.
            cinA = dram.tile([WSH, D], bf16, name="cinA")
            coutA = dram.tile([D, D], bf16, name="coutA")
            cinV = dram.tile([WSH, D], bf16, name="cinV")
            coutV = dram.tile([D, D], bf16, name="coutV")
            nc.gpsimd.dma_start(cinA[:], blob_d.ap()[T : T + WSH, :])
            nc.gpsimd.dma_start(cinV[:], blob_d.ap()[T + WSH : T + 2 * WSH, :])
            # A.T gathers first so stage A1 can start ~28us in; the Avo
            # gather's latency hides under A1+B (A2 runs after B).
            for cc_in, cc_out in ((cinA, coutA), (cinV, coutV)):
                nc.gpsimd.collective_compute(
                    "AllGather",
                    mybir.AluOpType.bypass,
                    replica_groups=[list(range(B))],
                    ins=[cc_in.opt()],
                    outs=[cc_out.opt()],
                )
            nc.sync.dma_start(ident[:], ident_d.ap())
            nc.sync.dma_start(
                kbias16[:],
                blob_d.ap()[T + 2 * WSH : T + 2 * WSH + 4, :].rearrange(
                    "a (q n) -> (a q) n", n=NT
                ),
            )
            nc.vector.tensor_copy(kbias[:], kbias16[:])
            nc.sync.dma_start(
                xrow[:], blob_d.ap()[0:T, :].rearrange("(n p) d -> p n d", p=P)
            )
            nc.sync.dma_start(AT[:], coutA.rearrange("(c p) h -> p c h", p=P))
            nc.sync.dma_start(Avo[:], coutV.rearrange("(c p) h -> p c h", p=P))
            nc.vector.memset(dacc[:], 0.0)

            # ---- stage T: xT = x^T via PE transpose, 4 blocks per copy ----
            for c in range(KC):
                for nb in range(0, NT, 4):
                    pt = psumt.tile([P, 4 * P], bf16, tag="pt", name="pt")
                    for i in range(4):
                        nc.tensor.transpose(
                            pt[:, i * P : (i + 1) * P],
                            xrow[:, nb + i, c * P : (c + 1) * P],
                            ident[:],
                        )
                    nc.vector.tensor_copy(
                        xT[:, c, nb * P : (nb + 4) * P], pt[:]
                    )

            # ---- stage A1: u = A @ x^T  [x, j] ----
            for jc in range(NQ):
                pk = [psum.tile([P, QB], f32, tag="ps", name="ps")
                      for _ in range(KC)]
                for c in range(KC):
                    for m in range(KC):
                        nc.tensor.matmul(
                            pk[m][:],
                            AT[:, c, m * P : (m + 1) * P],
                            xT[:, c, jc * QB : (jc + 1) * QB],
                            start=(c == 0),
                            stop=(c == KC - 1),
                        )
                for m in range(KC):
                    nc.vector.tensor_copy(
                        u[:, m, jc * QB : (jc + 1) * QB], pk[m][:]
                    )

            # ---- stage B: scores + exp + denominator accumulation ----
            for j in range(NT):
                ps = [psum.tile([P, QB], f32, tag="ps", name="ps")
                      for _ in range(NQ)]
                for c in range(KC):
                    for t in range(NQ):
                        nc.tensor.matmul(
                            ps[t][:],
                            u[:, c, j * P : (j + 1) * P],
                            xT[:, c, t * QB : (t + 1) * QB],
                            start=(c == 0),
                            stop=(c == KC - 1),
                        )
                for t in range(NQ):
                    sl = slice(t * QB, (t + 1) * QB)
                    nc.scalar.activation(
                        eT[:, j, sl],
                        ps[t][:],
                        Exp,
                        bias=kbias[:, j : j + 1],
                        scale=SCALE,
                    )
                    nc.vector.tensor_add(dacc[:, sl], dacc[:, sl], eT[:, j, sl])

            # ---- stage A2: v2 = x @ Avo  [j, o] ----
            for j in range(NT):
                pv = psum.tile([P, D], f32, tag="ps", name="ps")
                for c in range(KC):
                    nc.tensor.matmul(
                        pv[:],
                        xT[:, c, j * P : (j + 1) * P],
                        Avo[:, c, :],
                        start=(c == 0),
                        stop=(c == KC - 1),
                    )
                nc.vector.tensor_copy(v2[:, j, :], pv[:])

            # ---- denominator: gpsimd all-reduce across partitions (runs
            # concurrently with stage C on the PE); the raw sums ship to the
            # host, which folds 1/den into the dequant scale.
            for tt in range(NQ):
                sl = slice(tt * QB, (tt + 1) * QB)
                nc.gpsimd.partition_all_reduce(
                    rbc[:, sl], dacc[:, sl], P, bass_isa.ReduceOp.add
                )
            for k in range(NT):
                nc.sync.dma_start(
                    out_d[T + NT + k : T + NT + k + 1, :].bitcast(f32),
                    rbc[0:1, k * P : (k + 1) * P],
                )

            # ---- stage C: out[t,o] = sum_j e[j,t] v2[j,o] (unnormalized) in natural
            # orientation (lhsT = eT chunk, rhs = v2); each [128, D] tile is
            # quantized to int8 with a per-row (per query) scale and streams
            # to DRAM as soon as it completes.
            scl = bpool.tile([P, NT], f32, tag="scl")
            for tt in range(NT):
                po = psum.tile([P, D], f32, tag="ps", name="ps")
                for j in range(NT):
                    nc.tensor.matmul(
                        po[:],
                        eT[:, j, tt * P : (tt + 1) * P],
                        v2[:, j, :],
                        start=(j == 0),
                        stop=(j == NT - 1),
                    )
                mx = opool.tile([P, 1], f32, tag="mx", name="mx")
                rq = opool.tile([P, 1], f32, tag="rq", name="rq")
                nc.vector.tensor_reduce(
                    mx[:], po[:], mybir.AxisListType.X,
                    mybir.AluOpType.max, apply_absolute_value=True,
                )
                nc.vector.tensor_scalar_max(mx[:], mx[:], 1e-30)
                nc.vector.reciprocal(rq[:], mx[:])
                nc.vector.tensor_scalar_mul(rq[:], rq[:], 127.0)
                nc.vector.tensor_scalar_mul(
                    scl[:, tt : tt + 1], mx[:], 1.0 / 127.0
                )
                ot = opool.tile([P, D], i8, tag="ot", name="ot")
                nc.vector.tensor_mul(ot[:], po[:], rq[:].broadcast_to([P, D]))
                nc.sync.dma_start(out_d[tt * P : (tt + 1) * P, :], ot[:])
            # scales: [P, NT] f32 = 64 bytes/partition -> 16 int8 rows
            nc.sync.dma_start(
                out_d[T : T + NT, :].rearrange("a (q m) -> (a q) m", m=64),
                scl[:].bitcast(i8),
            )

    nc.compile()
    return nc


def _get_ctx():
    """Build the program and a cached jitted executable (once per process)."""
    if "run" in _ctx:
        return _ctx
    import jax
    import jax.numpy as jnp
    from jax.experimental.shard_map import shard_map
    from jax.sharding import Mesh, PartitionSpec, NamedSharding
    import concourse.mybir as mybir
    from concourse import bass2jax

    bass2jax.install_neuronx_cc_hook()
    nc = _build()
    partition_name = nc.partition_id_tensor.name if nc.partition_id_tensor else None
    in_names, out_names, out_avals = [], [], []
    for alloc in nc.m.functions[0].allocations:
        if not isinstance(alloc, mybir.MemoryLocationSet):
            continue
        name = alloc.memorylocations[0].name
        if alloc.kind == "ExternalInput":
            if name != partition_name:
                in_names.append(name)
        elif alloc.kind == "ExternalOutput":
            out_names.append(name)
            shape = tuple(alloc.tensor_shape)
            dtype = mybir.dt.np(alloc.dtype)
            out_avals.append(jax.core.ShapedArray(shape, dtype))
    n_params = len(in_names)
    n_outs = len(out_avals)
    all_names = list(in_names) + out_names
    if partition_name is not None:
        all_names = all_names + [partition_name]
    donate = tuple(range(n_params, n_params + n_outs))

    def _body(*args):
        operands = list(args)
        if partition_name is not None:
            operands.append(bass2jax.partition_id_tensor())
        outs = bass2jax._bass_exec_p.bind(
            *operands,
            out_avals=tuple(out_avals),
            in_names=tuple(all_names),
            out_names=tuple(out_names),
            lowering_input_output_aliases=(),
            sim_require_finite=True,
            sim_require_nnan=True,
            nc=nc,
        )
        return tuple(outs)

    devices = jax.devices()[:B]
    mesh = Mesh(np.asarray(devices), ("core",))
    in_specs = (PartitionSpec("core"),) * (n_params + n_outs)
    out_specs = (PartitionSpec("core"),) * n_outs
    sharded = jax.jit(
        shard_map(_body, mesh=mesh, in_specs=in_specs, out_specs=out_specs,
                  check_rep=False),
        donate_argnums=donate,
        keep_unused=True,
    )

    csh = NamedSharding(mesh, PartitionSpec("core"))
    zero_fn = jax.jit(
        lambda: tuple(
            jnp.zeros((B * a.shape[0],) + tuple(a.shape[1:]), a.dtype)
            for a in out_avals
        ),
        out_shardings=(csh,) * n_outs,
    )

    # identity matrix is a constant input: keep it resident on device.
    ident_np = np.tile(np.eye(P, dtype=np.float32).astype(_BF16), (B, 1))
    ident_dev = jax.device_put(ident_np, csh)

    _ctx.update(
        nc=nc,
        in_names=in_names,
        sharded=sharded,
        zero_fn=zero_fn,
        ident_dev=ident_dev,
        prev_out=None,
        run=True,
    )
    return _ctx


def _prep_args(x, mask, W_q, W_k, W_v, W_o):
    """Host-side prep: one bf16 blob per core (x, weight shards, key bias)."""
    R = T + 2 * WSH + 4
    blob = _ctx.get("blob_buf")
    if blob is None:
        blob = np.empty((B, R, D), _BF16)
        _ctx["blob_buf"] = blob
    np.copyto(blob[:, :T, :], np.asarray(x), casting="unsafe")
    wq = np.asarray(W_q, np.float32)
    wk = np.asarray(W_k, np.float32)
    wv = np.asarray(W_v, np.float32)
    wo = np.asarray(W_o, np.float32)
    a = wq @ wk.T          # [x, x']; score = x @ A @ x^T
    avo = wv @ wo          # [x, o];  out = attn @ x @ Avo
    np.copyto(blob[:, T : T + WSH, :],
              a.T.reshape(B, WSH, D), casting="unsafe")
    np.copyto(blob[:, T + WSH : T + 2 * WSH, :],
              avo.reshape(B, WSH, D), casting="unsafe")
    # key bias, laid out [P, NT] row-major then packed into 4 rows of 512
    bias = np.where(np.asarray(mask) != 0, np.float32(0.0),
                    np.float32(PAD_BIAS)).astype(np.float32)      # [B, T]
    kb = bias.reshape(B, NT, P).transpose(0, 2, 1)                # [B, P, NT]
    np.copyto(blob[:, T + 2 * WSH :, :],
              kb.reshape(B, 4, D), casting="unsafe")
    return {"blob": blob.reshape(B * R, D)}


def kernel(x, mask, W_q, W_k, W_v, W_o):
    ctx = _get_ctx()
    args = _prep_args(x, mask, W_q, W_k, W_v, W_o)
    operands = []
    for name in ctx["in_names"]:
        if name == "ident":
            operands.append(ctx["ident_dev"])
        else:
            operands.append(args[name])
    try:
        if ctx["prev_out"] is not None:
            zeros = (ctx["prev_out"],)
        else:
            zeros = ctx["zero_fn"]()
        outs = ctx["sharded"](*operands, *zeros)
        # per-shard pull pipelined with host dequantization: shard i+1
        # streams over the tunnel while shard i is dequantized on the CPU.
        shards = sorted(
            outs[0].addressable_shards,
            key=lambda s: s.index[0].start if s.index[0].start else 0,
        )
        for s in shards:
            s.data.copy_to_host_async()
        res = np.empty((B, T, D), np.float32)
        for i, s in enumerate(shards):
            raw = np.asarray(s.data).reshape(T + 2 * NT, D)  # int8 rows
            # absmax/127 scales: 16 int8 rows -> f32 [128, 16], t = n*128+p;
            # denominators: 16 more rows, f32 in linear t order.
            scale = (
                np.ascontiguousarray(raw[T : T + NT, :])
                .view(np.float32)
                .reshape(P, NT)
                .T.reshape(T, 1)
            )
            den = (
                np.ascontiguousarray(raw[T + NT :, :])
                .view(np.float32)
                .reshape(T, 1)
            )
            np.multiply(raw[:T, :], scale / den, dtype=np.float32, out=res[i])
    except Exception:
        # don't let a failed call leave a donated/invalidated buffer cached
        ctx["prev_out"] = None
        raise
    ctx["prev_out"] = outs[0]
    return res



# revision 30
# speedup vs baseline: 2.5081x; 2.5081x over previous
# Trainium2 Bass kernel for masked (key-padding) attention layer.
#
#   q,k,v = x@Wq, x@Wk, x@Wv ; score = q@k^T/sqrt(T) masked over keys;
#   out = softmax(score)@v @ Wo
#
# Sharding: data-parallel over batch, B=8 -> one batch element per NeuronCore.
#
# This is the sparse_attention problem: the key-padding mask kills ~half the
# keys (mask ~ Bernoulli(0.5)), and masked keys contribute exactly nothing
# (exp(-inf) = 0).  The host therefore COMPACTS the keys per batch element
# (gathers the rows with mask=1, padded to K = KT*128 with zero rows whose
# bias is -30000), which cuts every key-dimension GEMM to ~K/T = ~9/16 of the
# dense cost.  The device is a pure GEMM pipeline:
#
#   u[x,j]  = sum_x' A[x,x'] xkT[x',j]   A = 64*Wq@Wk^T folded on host  (A1)
#   v2[j,o] = sum_x xkT[x,j] Avo[x,o]    Avo = 64*Wv@Wo folded on host  (A2)
#   sT[j,t] = sum_x u[x,j] xT[x,t]                                      (B)
#   eT      = exp(sT/(64 sqrt(T)) + kbias)   ScalarE, PSUM->SBUF bf16
#   den[t]  = 64 * sum_j eT[j,t]         PE matmul against a 64s vector
#   out[t,o]= (sum_j eT[j,t] v2[j,o]) / den[t]   (C + ScalarE scale, bf16)
#
# A1/A2/B run as RESIDUAL-FP8 DoubleRow matmuls: each operand is split into
# an fp8e4 value plus an fp8e4 residual (v = v8 + vr with |vr| <~ 6% |v|),
# and each product takes 3 of the 4 cross terms (v8*w8 + v8*wr + vr*w8,
# dropping the ~0.4% vr*wr term) accumulated in one PSUM group.  DoubleRow
# processes 2 rows/cycle, so 3 fp8 passes = 0.75x the bf16 cost with ~1e-3
# relative error.  The x/xk/A/Avo splits are precomputed on the host (free);
# u's split costs one ScalarE copy + one DVE subtract per PSUM tile.  The
# x64 pre-scale on A/Avo keeps their N(0, 1/512) entries out of the fp8e4
# subnormal range; it is folded back via the exp scale and the 64s vector.
#
# Everything is laid out by the HOST so the device does no transposes and no
# collectives; stage C stays bf16 (exp values would need fp8 splits that
# saturate the ScalarE).  The PE is the bottleneck engine (~65us of matmul);
# a short zeros-matmul warmup keeps it busy (and its p-state ramped) under
# the initial input DMA so it never idles: any PE gap >100ns re-triggers the
# 3us mid-speed ramp.
import math

import numpy as np
import ml_dtypes

B = 8
T = 2048
D = 512
P = 128
KC = D // P       # 4 contraction chunks of 128
KG = KC // 2      # 2 double-row groups
QB = 512          # free-dim chunk (one PSUM bank of f32)
NQ = T // QB      # 4 query chunks
NT = T // P       # 16 query tiles of 128
WSCALE = 64.0
SCALE = 1.0 / math.sqrt(float(T)) / WSCALE
PAD_BIAS = -30000.0
NWARM = 9         # zeros-matmul PE warmup instructions

_BF16 = ml_dtypes.bfloat16
_FP8 = ml_dtypes.float8_e4m3

# blob row offsets (512-wide fp8 rows); value/residual planes interleaved
# per partition so each logical tensor is ONE dma (HWDGE issue is ~630ns
# per dma, serialized, so fewer+larger transfers win).
ATB0 = 0                    # fp8 (64 A^T | resid)  [128,2,4,512] -> 1024 rows
AVB0 = 1024                 # fp8 (64 Avo | resid)  [128,2,4,512] -> 1024 rows
XTB0 = 2048                 # fp8 (xT | resid)      [128,2,4,2048] -> 4096 rows
RBLOB8 = 6144

_ctx: dict = {}


def _build(kt: int):
    """Build + compile the single-core SPMD program for KT=kt key tiles."""
    import concourse.bass as bass
    import concourse.mybir as mybir
    import concourse.tile as tile
    from concourse import bacc

    dt = mybir.dt
    f32, bf16, fp8 = dt.float32, dt.bfloat16, dt.float8e4
    K = kt * P

    nc = bacc.Bacc(
        "TRN2",
        target_bir_lowering=False,
        debug=False,
        enable_asserts=False,
        num_devices=B,
    )

    blob_d = nc.dram_tensor("blob", [RBLOB8, D], fp8, kind="ExternalInput")
    # compacted+transposed keys (value + residual), rows (p, c) so any
    # key-range slice is a clean per-row run: row p*4+c holds xk[:, c*128+p].
    xkb_d = nc.dram_tensor("xkb", [P * 2 * KC, K], fp8, kind="ExternalInput")
    kb_d = nc.dram_tensor("kb", [P, 16], f32, kind="ExternalInput")
    out_d = nc.dram_tensor("out", [T, D], bf16, kind="ExternalOutput")

    Exp = mybir.ActivationFunctionType.Exp
    DR = mybir.MatmulPerfMode.DoubleRow

    # key-chunk boundaries for A1 / the xk DMA split
    kchunks = []
    k0 = 0
    while k0 < K:
        kchunks.append((k0, min(k0 + QB, K)))
        k0 += QB

    with tile.TileContext(nc) as tc:
        with (
            tc.tile_pool(name="const", bufs=1) as cpool,
            tc.tile_pool(name="big", bufs=1) as bpool,
            tc.tile_pool(name="psum", bufs=8, space="PSUM") as psum,
            tc.tile_pool(name="outs", bufs=2) as opool,
            tc.tile_pool(name="small", bufs=4) as spool,
        ):
            # ---- persistent SBUF tensors ----
            xTB = bpool.tile([P, 2, KC, T], fp8, tag="xTB")
            xkB = bpool.tile([P, 2, KC, K], fp8, tag="xkB")
            ATB = cpool.tile([P, 2, KC, D], fp8, tag="ATB")
            AVB = cpool.tile([P, 2, KC, D], fp8, tag="AVB")
            kbias = cpool.tile([P, 16], f32, tag="kbias")
            uB = bpool.tile([P, 2, KC, K], fp8, tag="uB")
            v2 = bpool.tile([P, kt, D], bf16, tag="v2")
            eT = bpool.tile([P, kt, T], bf16, tag="eT")
            zeros = cpool.tile([P, QB], bf16, tag="zeros")
            ones = cpool.tile([P, 1], bf16, tag="ones")

            # ---- PE warmup feed + 64s vector for the denominator ----
            nc.vector.memset(zeros[:], 0.0)
            nc.vector.memset(ones[:], WSCALE)

            # ---- input DMAs, in critical-path order ----
            # the A-matrix and first key chunk arrive value-plane first:
            # mm3's pass order is (v,v),(v,r),(r,v), so the first matmuls
            # need only the value planes and start ~1.3us earlier.
            atb_src = blob_d.ap()[ATB0 : ATB0 + 2 * KC * P, :].rearrange(
                "(p v c) w -> p v c w", p=P, v=2
            )
            xk0_src = xkb_d.ap()[:, 0 : kchunks[0][1]].rearrange(
                "(p v c) k -> p v c k", p=P, v=2
            )
            nc.sync.dma_start(ATB[:, 0], atb_src[:, 0])
            nc.sync.dma_start(xkB[:, 0, :, 0 : kchunks[0][1]], xk0_src[:, 0])
            nc.sync.dma_start(xkB[:, 1, :, 0 : kchunks[0][1]], xk0_src[:, 1])
            nc.sync.dma_start(ATB[:, 1], atb_src[:, 1])
            for j0, j1 in kchunks[1:]:
                nc.sync.dma_start(
                    xkB[:, :, :, j0:j1],
                    xkb_d.ap()[:, j0:j1].rearrange(
                        "(p v c) k -> p v c k", p=P, v=2
                    ),
                )
            nc.sync.dma_start(
                AVB[:],
                blob_d.ap()[AVB0 : AVB0 + 2 * KC * P, :].rearrange(
                    "(p v c) w -> p v c w", p=P, v=2
                ),
            )
            nc.sync.dma_start(kbias[:], kb_d.ap())
            nc.sync.dma_start(
                xTB[:],
                blob_d.ap()[XTB0 : XTB0 + 4096, :].rearrange(
                    "(p v c r) w -> p v c (r w)", p=P, v=2, c=KC
                ),
            )

            # ---- PE warmup: ramp the p-state under the input DMAs ----
            wt = psum.tile([P, QB], f32, tag="ps", name="ps")
            for _ in range(NWARM):
                nc.tensor.matmul(
                    wt[:], zeros[:, 0:P], zeros[:], start=True, stop=True
                )

            def mm3(out, LT, RT, lsl, rsl):
                """3-cross-term residual-fp8 product into one PSUM group:
                value*value + value*resid + resid*value."""
                n = 0
                for lv, rv in ((0, 0), (0, 1), (1, 0)):
                    for g in range(KG):
                        gs = slice(2 * g, 2 * g + 2)
                        nc.tensor.matmul(
                            out,
                            LT[:, lv, gs, lsl],
                            RT[:, rv, gs, rsl],
                            start=(n == 0),
                            stop=(n == 3 * KG - 1),
                            perf_mode=DR,
                        )
                        n += 1

            # ---- stage A1: u[x, j] = (64 A) @ xkT, then split u -> u8+ur ----
            for j0, j1 in kchunks:
                cw = j1 - j0
                for m in range(KC):
                    pk = psum.tile([P, QB], f32, tag="ps", name="ps")
                    mm3(pk[:, :cw], ATB, xkB,
                        slice(m * P, (m + 1) * P), slice(j0, j1))
                    nc.scalar.copy(uB[:, 0, m, j0:j1], pk[:, :cw])
                    nc.vector.tensor_sub(
                        uB[:, 1, m, j0:j1], pk[:, :cw], uB[:, 0, m, j0:j1]
                    )

            # ---- stage A2: v2[j, o] = xk @ (64 Avo), bf16 result ----
            for j in range(kt):
                pv = psum.tile([P, D], f32, tag="ps", name="ps")
                mm3(pv[:], xkB, AVB,
                    slice(j * P, (j + 1) * P), slice(0, D))
                nc.scalar.copy(v2[:, j, :], pv[:])

            # ---- stages B (scores+exp) and C (output), pipelined per
            # 512-query chunk so C consumes eT while B fills the next chunk.
            for tq in range(NQ):
                sl = slice(tq * QB, (tq + 1) * QB)
                for j in range(kt):
                    ps = psum.tile([P, QB], f32, tag="ps", name="ps")
                    mm3(ps[:], uB, xTB,
                        slice(j * P, (j + 1) * P), sl)
                    nc.scalar.activation(
                        eT[:, j, sl],
                        ps[:],
                        Exp,
                        bias=kbias[:, j : j + 1],
                        scale=SCALE,
                    )
                for q in range(NQ):
                    tt = tq * NQ + q
                    tsl = slice(tt * P, (tt + 1) * P)
                    # denominator first: the DVE reciprocal overlaps the
                    # C-tile matmuls that follow.
                    dps = psum.tile([P, 1], f32, tag="ps", name="ps")
                    for j in range(kt):
                        nc.tensor.matmul(
                            dps[:],
                            eT[:, j, tsl],
                            ones[:],
                            start=(j == 0),
                            stop=(j == kt - 1),
                        )
                    rp = spool.tile([P, 1], f32, tag="rp", name="rp")
                    nc.vector.reciprocal(rp[:], dps[:])
                    po = psum.tile([P, D], f32, tag="ps", name="ps")
                    for j in range(kt):
                        nc.tensor.matmul(
                            po[:],
                            eT[:, j, tsl],
                            v2[:, j, :],
                            start=(j == 0),
                            stop=(j == kt - 1),
                        )
                    osb = opool.tile([P, QB], bf16, tag="osb", name="osb")
                    nc.scalar.mul(osb[:], po[:], rp[:])
                    nc.sync.dma_start(
                        out_d.ap()[tt * P : (tt + 1) * P, :], osb[:]
                    )

    nc.compile()
    return nc


def _get_ctx(kt: int):
    """Build the program and a cached jitted executable (once per KT)."""
    if kt in _ctx:
        return _ctx[kt]
    import jax
    import jax.numpy as jnp
    from jax.experimental.shard_map import shard_map
    from jax.sharding import Mesh, PartitionSpec, NamedSharding
    import concourse.mybir as mybir
    from concourse import bass2jax

    bass2jax.install_neuronx_cc_hook()
    nc = _build(kt)
    partition_name = nc.partition_id_tensor.name if nc.partition_id_tensor else None
    in_names, out_names, out_avals = [], [], []
    for alloc in nc.m.functions[0].allocations:
        if not isinstance(alloc, mybir.MemoryLocationSet):
            continue
        name = alloc.memorylocations[0].name
        if alloc.kind == "ExternalInput":
            if name != partition_name:
                in_names.append(name)
        elif alloc.kind == "ExternalOutput":
            out_names.append(name)
            shape = tuple(alloc.tensor_shape)
            dtype = mybir.dt.np(alloc.dtype)
            out_avals.append(jax.core.ShapedArray(shape, dtype))
    n_params = len(in_names)
    n_outs = len(out_avals)
    all_names = list(in_names) + out_names
    if partition_name is not None:
        all_names = all_names + [partition_name]
    donate = tuple(range(n_params, n_params + n_outs))

    def _body(*args):
        operands = list(args)
        if partition_name is not None:
            operands.append(bass2jax.partition_id_tensor())
        outs = bass2jax._bass_exec_p.bind(
            *operands,
            out_avals=tuple(out_avals),
            in_names=tuple(all_names),
            out_names=tuple(out_names),
            lowering_input_output_aliases=(),
            sim_require_finite=True,
            sim_require_nnan=True,
            nc=nc,
        )
        return tuple(outs)

    devices = jax.devices()[:B]
    mesh = Mesh(np.asarray(devices), ("core",))
    in_specs = (PartitionSpec("core"),) * (n_params + n_outs)
    out_specs = (PartitionSpec("core"),) * n_outs
    sharded = jax.jit(
        shard_map(_body, mesh=mesh, in_specs=in_specs, out_specs=out_specs,
                  check_rep=False),
        donate_argnums=donate,
        keep_unused=True,
    )

    csh = NamedSharding(mesh, PartitionSpec("core"))
    zero_fn = jax.jit(
        lambda: tuple(
            jnp.zeros((B * a.shape[0],) + tuple(a.shape[1:]), a.dtype)
            for a in out_avals
        ),
        out_shardings=(csh,) * n_outs,
    )

    ctx = dict(
        nc=nc,
        in_names=in_names,
        sharded=sharded,
        zero_fn=zero_fn,
        prev_out=None,
    )
    _ctx[kt] = ctx
    return ctx


def _f8split(a):
    """fp8 value + fp8 residual of a float32 array."""
    v = a.astype(_FP8)
    r = (a - v.astype(np.float32)).astype(_FP8)
    return v, r


def _prep_args(x, mask, W_q, W_k, W_v, W_o, kt: int):
    """Host-side prep: fp8 value/residual splits in device SBUF layouts."""
    K = kt * P
    x = np.asarray(x, np.float32)
    mask = np.asarray(mask)
    blob = np.empty((B, RBLOB8, D), _FP8)
    xkb = np.zeros((B, P, 2, KC, K), _FP8)

    wq = np.asarray(W_q, np.float32)
    wk = np.asarray(W_k, np.float32)
    wv = np.asarray(W_v, np.float32)
    wo = np.asarray(W_o, np.float32)
    a = (wq @ wk.T) * WSCALE    # [x, x']; score = x @ A @ x^T / 64
    avo = (wv @ wo) * WSCALE    # [x, o];  64 * attn-value product
    at = a.T.reshape(KC, P, D).transpose(1, 0, 2)      # [128, 4, 512]
    av = avo.reshape(KC, P, D).transpose(1, 0, 2)
    atb = np.stack(_f8split(at), axis=1)               # [128, 2, 4, 512]
    avb = np.stack(_f8split(av), axis=1)
    blob[:, ATB0 : ATB0 + 2 * KC * P] = atb.reshape(2 * KC * P, D)
    blob[:, AVB0 : AVB0 + 2 * KC * P] = avb.reshape(2 * KC * P, D)

    # xT value/residual planes: [128, 2, 4, 2048] -> 32 rows per partition
    xt = np.ascontiguousarray(x.reshape(B, T, KC, P).transpose(0, 3, 2, 1))
    xtb = np.stack(_f8split(xt), axis=2)               # [B, 128, 2, 4, 2048]
    blob[:, XTB0 : XTB0 + 4096] = xtb.reshape(B, 4096, D)

    # per-core key compaction + bias
    kb = np.full((B, P, 16), PAD_BIAS, np.float32)
    for b in range(B):
        idx = np.flatnonzero(mask[b])
        kn = len(idx)
        xkt = x[b][idx].reshape(kn, KC, P).transpose(2, 1, 0)  # [128, 4, kn]
        v, r = _f8split(np.ascontiguousarray(xkt))
        xkb[b, :, 0, :, :kn] = v
        xkb[b, :, 1, :, :kn] = r
        jt, pp = np.divmod(np.arange(kn), P)
        kb[b, pp, jt] = 0.0
    return {
        "blob": blob.reshape(B * RBLOB8, D),
        "xkb": xkb.reshape(B * P * 2 * KC, K),
        "kb": kb.reshape(B * P, 16),
    }


def kernel(x, mask, W_q, W_k, W_v, W_o):
    mask = np.asarray(mask)
    counts = (mask != 0).sum(axis=1)
    kt = max(1, int(-(-int(counts.max()) // P)))
    ctx = _get_ctx(kt)
    args = _prep_args(x, mask, W_q, W_k, W_v, W_o, kt)
    operands = [args[name] for name in ctx["in_names"]]
    try:
        if ctx["prev_out"] is not None:
            zeros = (ctx["prev_out"],)
        else:
            zeros = ctx["zero_fn"]()
        outs = ctx["sharded"](*operands, *zeros)
        shards = sorted(
            outs[0].addressable_shards,
            key=lambda s: s.index[0].start if s.index[0].start else 0,
        )
        for s in shards:
            s.data.copy_to_host_async()
        res = np.empty((B, T, D), np.float32)
        for i, s in enumerate(shards):
            res[i] = np.asarray(s.data).astype(np.float32)
    except Exception:
        ctx["prev_out"] = None
        raise
    ctx["prev_out"] = outs[0]
    return res


# revision 33
# speedup vs baseline: 2.7848x; 1.1104x over previous
# Trainium2 Bass kernel for masked (key-padding) attention layer.
#
#   q,k,v = x@Wq, x@Wk, x@Wv ; score = q@k^T/sqrt(T) masked over keys;
#   out = softmax(score)@v @ Wo
#
# Sharding: data-parallel over batch, B=8 -> one batch element per NeuronCore.
#
# This is the sparse_attention problem: the key-padding mask kills ~half the
# keys (mask ~ Bernoulli(0.5)), and masked keys contribute exactly nothing
# (exp(-inf) = 0).  The host therefore COMPACTS the keys per batch element
# (gathers the rows with mask=1, padded to K = KT*128 with zero rows whose
# bias is -30000), which cuts every key-dimension GEMM to ~K/T = ~9/16 of the
# dense cost.  The device is a pure GEMM pipeline:
#
#   u[x,j]  = sum_x' A[x,x'] xkT[x',j]   A = 64*Wq@Wk^T folded on host  (A1)
#   v2[j,o] = sum_x xkT[x,j] Avo[x,o]    Avo = 64*Wv@Wo folded on host  (A2)
#   sT[j,t] = sum_x u[x,j] xT[x,t]                                      (B)
#   eT      = exp(sT/(64 sqrt(T)) + kbias)   ScalarE, PSUM->SBUF bf16
#   den[t]  = 64 * sum_j eT[j,t]         PE matmul against a 64s vector
#   out[t,o]= (sum_j eT[j,t] v2[j,o]) / den[t]   (C + ScalarE scale, bf16)
#
# A1/A2/B run as RESIDUAL-FP8 DoubleRow matmuls: each operand is split into
# an fp8e4 value plus an fp8e4 residual (v = v8 + vr with |vr| <~ 6% |v|),
# and each product takes 3 of the 4 cross terms (v8*w8 + v8*wr + vr*w8,
# dropping the ~0.4% vr*wr term) accumulated in one PSUM group.  DoubleRow
# processes 2 rows/cycle, so 3 fp8 passes = 0.75x the bf16 cost with ~1e-3
# relative error.  The x/xk/A/Avo splits are precomputed on the host (free);
# u's split costs one ScalarE copy + one DVE subtract per PSUM tile.  The
# x64 pre-scale on A/Avo keeps their N(0, 1/512) entries out of the fp8e4
# subnormal range; it is folded back via the exp scale and the 64s vector.
#
# Everything is laid out by the HOST so the device does no transposes and no
# collectives; stage C stays bf16 (exp values would need fp8 splits that
# saturate the ScalarE).  The PE is the bottleneck engine (~65us of matmul);
# a short zeros-matmul warmup keeps it busy (and its p-state ramped) under
# the initial input DMA so it never idles: any PE gap >100ns re-triggers the
# 3us mid-speed ramp.
import math

import numpy as np
import ml_dtypes

B = 8
T = 2048
D = 512
P = 128
KC = D // P       # 4 contraction chunks of 128
KG = KC // 2      # 2 double-row groups
QB = 512          # free-dim chunk (one PSUM bank of f32)
NQ = T // QB      # 4 query chunks
NT = T // P       # 16 query tiles of 128
WSCALE = 64.0
SCALE = 1.0 / math.sqrt(float(T)) / WSCALE
PAD_BIAS = -30000.0
NWARM = 9         # zeros-matmul PE warmup instructions

_BF16 = ml_dtypes.bfloat16
_FP8 = ml_dtypes.float8_e4m3

# blob row offsets (512-wide fp8 rows); value/residual planes interleaved
# per partition so each logical tensor is ONE dma (HWDGE issue is ~630ns
# per dma, serialized, so fewer+larger transfers win).
ATB0 = 0                    # fp8 (64 A^T | resid)  [128,2,4,512] -> 1024 rows
AVB0 = 1024                 # fp8 (64 Avo | resid)  [128,2,4,512] -> 1024 rows
XTB0 = 2048                 # fp8 (xT | resid)      [128,2,4,2048] -> 4096 rows
RBLOB8 = 6144

_ctx: dict = {}


def _build(kt: int):
    """Build + compile the single-core SPMD program for KT=kt key tiles."""
    import concourse.bass as bass
    import concourse.mybir as mybir
    import concourse.tile as tile
    from concourse import bacc

    dt = mybir.dt
    f32, bf16, fp8 = dt.float32, dt.bfloat16, dt.float8e4
    K = kt * P

    nc = bacc.Bacc(
        "TRN2",
        target_bir_lowering=False,
        debug=False,
        enable_asserts=False,
        num_devices=B,
    )

    blob_d = nc.dram_tensor("blob", [RBLOB8, D], fp8, kind="ExternalInput")
    # compacted+transposed keys (value + residual), rows (p, c) so any
    # key-range slice is a clean per-row run: row p*4+c holds xk[:, c*128+p].
    xkb_d = nc.dram_tensor("xkb", [P * 2 * KC, K], fp8, kind="ExternalInput")
    kb_d = nc.dram_tensor("kb", [P, 16], f32, kind="ExternalInput")
    out_d = nc.dram_tensor("out", [T, D], bf16, kind="ExternalOutput")

    Exp = mybir.ActivationFunctionType.Exp
    DR = mybir.MatmulPerfMode.DoubleRow

    # key-chunk boundaries for A1 / the xk DMA split
    kchunks = []
    k0 = 0
    while k0 < K:
        kchunks.append((k0, min(k0 + QB, K)))
        k0 += QB

    with tile.TileContext(nc) as tc:
        with (
            tc.tile_pool(name="const", bufs=1) as cpool,
            tc.tile_pool(name="big", bufs=1) as bpool,
            tc.tile_pool(name="psum", bufs=8, space="PSUM") as psum,
            tc.tile_pool(name="outs", bufs=2) as opool,
            tc.tile_pool(name="small", bufs=4) as spool,
        ):
            # ---- persistent SBUF tensors ----
            xTB = bpool.tile([P, 2, KC, T], fp8, tag="xTB")
            xkB = bpool.tile([P, 2, KC, K], fp8, tag="xkB")
            ATB = cpool.tile([P, 2, KC, D], fp8, tag="ATB")
            AVB = cpool.tile([P, 2, KC, D], fp8, tag="AVB")
            kbias = cpool.tile([P, 16], f32, tag="kbias")
            uB = bpool.tile([P, 2, KC, K], fp8, tag="uB")
            v2 = bpool.tile([P, kt, D], bf16, tag="v2")
            eT = bpool.tile([P, kt, T], bf16, tag="eT")
            zeros = cpool.tile([P, QB], bf16, tag="zeros")
            ones = cpool.tile([P, 1], bf16, tag="ones")

            # ---- PE warmup feed + 64s vector for the denominator ----
            nc.vector.memset(zeros[:], 0.0)
            nc.vector.memset(ones[:], WSCALE)

            # ---- input DMAs, in critical-path order ----
            # the A-matrix and first key chunk arrive value-plane first:
            # mm3's pass order is (v,v),(v,r),(r,v), so the first matmuls
            # need only the value planes and start ~1.3us earlier.
            atb_src = blob_d.ap()[ATB0 : ATB0 + 2 * KC * P, :].rearrange(
                "(p v c) w -> p v c w", p=P, v=2
            )
            xk0_src = xkb_d.ap()[:, 0 : kchunks[0][1]].rearrange(
                "(p v c) k -> p v c k", p=P, v=2
            )
            nc.sync.dma_start(ATB[:, 0], atb_src[:, 0])
            nc.sync.dma_start(xkB[:, 0, :, 0 : kchunks[0][1]], xk0_src[:, 0])
            nc.sync.dma_start(xkB[:, 1, :, 0 : kchunks[0][1]], xk0_src[:, 1])
            nc.sync.dma_start(ATB[:, 1], atb_src[:, 1])
            for j0, j1 in kchunks[1:]:
                nc.sync.dma_start(
                    xkB[:, :, :, j0:j1],
                    xkb_d.ap()[:, j0:j1].rearrange(
                        "(p v c) k -> p v c k", p=P, v=2
                    ),
                )
            nc.sync.dma_start(
                AVB[:],
                blob_d.ap()[AVB0 : AVB0 + 2 * KC * P, :].rearrange(
                    "(p v c) w -> p v c w", p=P, v=2
                ),
            )
            nc.sync.dma_start(kbias[:], kb_d.ap())
            nc.sync.dma_start(
                xTB[:],
                blob_d.ap()[XTB0 : XTB0 + 4096, :].rearrange(
                    "(p v c r) w -> p v c (r w)", p=P, v=2, c=KC
                ),
            )

            # ---- PE warmup: ramp the p-state under the input DMAs ----
            wt = psum.tile([P, QB], f32, tag="ps", name="ps")
            for _ in range(NWARM):
                nc.tensor.matmul(
                    wt[:], zeros[:, 0:P], zeros[:], start=True, stop=True
                )

            def mm3(out, LT, RT, lsl, rsl,
                    passes=((0, 0), (0, 1), (1, 0))):
                """residual-fp8 product into one PSUM group; default takes
                the 3 cross terms value*value + value*resid + resid*value."""
                n = 0
                for lv, rv in passes:
                    for g in range(KG):
                        gs = slice(2 * g, 2 * g + 2)
                        nc.tensor.matmul(
                            out,
                            LT[:, lv, gs, lsl],
                            RT[:, rv, gs, rsl],
                            start=(n == 0),
                            stop=(n == len(passes) * KG - 1),
                            perf_mode=DR,
                        )
                        n += 1

            # ---- stage A1: u[x, j] = (64 A) @ xkT, then split u -> u8+ur ----
            for j0, j1 in kchunks:
                cw = j1 - j0
                for m in range(KC):
                    pk = psum.tile([P, QB], f32, tag="ps", name="ps")
                    mm3(pk[:, :cw], ATB, xkB,
                        slice(m * P, (m + 1) * P), slice(j0, j1))
                    nc.scalar.copy(uB[:, 0, m, j0:j1], pk[:, :cw])

            # ---- stage A2: v2[j, o] = xk @ (64 Avo), bf16 result ----
            for j in range(kt):
                pv = psum.tile([P, D], f32, tag="ps", name="ps")
                mm3(pv[:], xkB, AVB,
                    slice(j * P, (j + 1) * P), slice(0, D))
                nc.scalar.copy(v2[:, j, :], pv[:])

            # ---- stages B (scores+exp) and C (output), pipelined per
            # 512-query chunk so C consumes eT while B fills the next chunk.
            for tq in range(NQ):
                sl = slice(tq * QB, (tq + 1) * QB)
                for j in range(kt):
                    ps = psum.tile([P, QB], f32, tag="ps", name="ps")
                    mm3(ps[:], uB, xTB,
                        slice(j * P, (j + 1) * P), sl,
                        passes=((0, 0), (0, 1)))
                    nc.scalar.activation(
                        eT[:, j, sl],
                        ps[:],
                        Exp,
                        bias=kbias[:, j : j + 1],
                        scale=SCALE,
                    )
                for q in range(NQ):
                    tt = tq * NQ + q
                    tsl = slice(tt * P, (tt + 1) * P)
                    # denominator first: the DVE reciprocal overlaps the
                    # C-tile matmuls that follow.
                    dps = psum.tile([P, 1], f32, tag="ps", name="ps")
                    for j in range(kt):
                        nc.tensor.matmul(
                            dps[:],
                            eT[:, j, tsl],
                            ones[:],
                            start=(j == 0),
                            stop=(j == kt - 1),
                        )
                    rp = spool.tile([P, 1], f32, tag="rp", name="rp")
                    nc.vector.reciprocal(rp[:], dps[:])
                    po = psum.tile([P, D], f32, tag="ps", name="ps")
                    for j in range(kt):
                        nc.tensor.matmul(
                            po[:],
                            eT[:, j, tsl],
                            v2[:, j, :],
                            start=(j == 0),
                            stop=(j == kt - 1),
                        )
                    osb = opool.tile([P, QB], bf16, tag="osb", name="osb")
                    nc.scalar.mul(osb[:], po[:], rp[:])
                    nc.sync.dma_start(
                        out_d.ap()[tt * P : (tt + 1) * P, :], osb[:]
                    )

    nc.compile()
    return nc


def _get_ctx(kt: int):
    """Build the program and a cached jitted executable (once per KT)."""
    if kt in _ctx:
        return _ctx[kt]
    import jax
    import jax.numpy as jnp
    from jax.experimental.shard_map import shard_map
    from jax.sharding import Mesh, PartitionSpec, NamedSharding
    import concourse.mybir as mybir
    from concourse import bass2jax

    bass2jax.install_neuronx_cc_hook()
    nc = _build(kt)
    partition_name = nc.partition_id_tensor.name if nc.partition_id_tensor else None
    in_names, out_names, out_avals = [], [], []
    for alloc in nc.m.functions[0].allocations:
        if not isinstance(alloc, mybir.MemoryLocationSet):
            continue
        name = alloc.memorylocations[0].name
        if alloc.kind == "ExternalInput":
            if name != partition_name:
                in_names.append(name)
        elif alloc.kind == "ExternalOutput":
            out_names.append(name)
            shape = tuple(alloc.tensor_shape)
            dtype = mybir.dt.np(alloc.dtype)
            out_avals.append(jax.core.ShapedArray(shape, dtype))
    n_params = len(in_names)
    n_outs = len(out_avals)
    all_names = list(in_names) + out_names
    if partition_name is not None:
        all_names = all_names + [partition_name]
    donate = tuple(range(n_params, n_params + n_outs))

    def _body(*args):
        operands = list(args)
        if partition_name is not None:
            operands.append(bass2jax.partition_id_tensor())
        outs = bass2jax._bass_exec_p.bind(
            *operands,
            out_avals=tuple(out_avals),
            in_names=tuple(all_names),
            out_names=tuple(out_names),
            lowering_input_output_aliases=(),
            sim_require_finite=True,
            sim_require_nnan=True,
            nc=nc,
        )
        return tuple(outs)

    devices = jax.devices()[:B]
    mesh = Mesh(np.asarray(devices), ("core",))
    in_specs = (PartitionSpec("core"),) * (n_params + n_outs)
    out_specs = (PartitionSpec("core"),) * n_outs
    sharded = jax.jit(
        shard_map(_body, mesh=mesh, in_specs=in_specs, out_specs=out_specs,
                  check_rep=False),
        donate_argnums=donate,
        keep_unused=True,
    )

    csh = NamedSharding(mesh, PartitionSpec("core"))
    zero_fn = jax.jit(
        lambda: tuple(
            jnp.zeros((B * a.shape[0],) + tuple(a.shape[1:]), a.dtype)
            for a in out_avals
        ),
        out_shardings=(csh,) * n_outs,
    )

    ctx = dict(
        nc=nc,
        in_names=in_names,
        sharded=sharded,
        zero_fn=zero_fn,
        prev_out=None,
    )
    _ctx[kt] = ctx
    return ctx


def _f8split(a):
    """fp8 value + fp8 residual of a float32 array."""
    v = a.astype(_FP8)
    r = (a - v.astype(np.float32)).astype(_FP8)
    return v, r


def _prep_args(x, mask, W_q, W_k, W_v, W_o, kt: int):
    """Host-side prep: fp8 value/residual splits in device SBUF layouts."""
    K = kt * P
    x = np.asarray(x, np.float32)
    mask = np.asarray(mask)
    blob = np.empty((B, RBLOB8, D), _FP8)
    xkb = np.zeros((B, P, 2, KC, K), _FP8)

    wq = np.asarray(W_q, np.float32)
    wk = np.asarray(W_k, np.float32)
    wv = np.asarray(W_v, np.float32)
    wo = np.asarray(W_o, np.float32)
    a = (wq @ wk.T) * WSCALE    # [x, x']; score = x @ A @ x^T / 64
    avo = (wv @ wo) * WSCALE    # [x, o];  64 * attn-value product
    at = a.T.reshape(KC, P, D).transpose(1, 0, 2)      # [128, 4, 512]
    av = avo.reshape(KC, P, D).transpose(1, 0, 2)
    atb = np.stack(_f8split(at), axis=1)               # [128, 2, 4, 512]
    avb = np.stack(_f8split(av), axis=1)
    blob[:, ATB0 : ATB0 + 2 * KC * P] = atb.reshape(2 * KC * P, D)
    blob[:, AVB0 : AVB0 + 2 * KC * P] = avb.reshape(2 * KC * P, D)

    # xT value/residual planes: [128, 2, 4, 2048] -> 32 rows per partition
    xt = np.ascontiguousarray(x.reshape(B, T, KC, P).transpose(0, 3, 2, 1))
    xtb = np.stack(_f8split(xt), axis=2)               # [B, 128, 2, 4, 2048]
    blob[:, XTB0 : XTB0 + 4096] = xtb.reshape(B, 4096, D)

    # per-core key compaction + bias
    kb = np.full((B, P, 16), PAD_BIAS, np.float32)
    for b in range(B):
        idx = np.flatnonzero(mask[b])
        kn = len(idx)
        xkt = x[b][idx].reshape(kn, KC, P).transpose(2, 1, 0)  # [128, 4, kn]
        v, r = _f8split(np.ascontiguousarray(xkt))
        xkb[b, :, 0, :, :kn] = v
        xkb[b, :, 1, :, :kn] = r
        jt, pp = np.divmod(np.arange(kn), P)
        kb[b, pp, jt] = 0.0
    return {
        "blob": blob.reshape(B * RBLOB8, D),
        "xkb": xkb.reshape(B * P * 2 * KC, K),
        "kb": kb.reshape(B * P, 16),
    }


def kernel(x, mask, W_q, W_k, W_v, W_o):
    mask = np.asarray(mask)
    counts = (mask != 0).sum(axis=1)
    kt = max(1, int(-(-int(counts.max()) // P)))
    ctx = _get_ctx(kt)
    args = _prep_args(x, mask, W_q, W_k, W_v, W_o, kt)
    operands = [args[name] for name in ctx["in_names"]]
    try:
        if ctx["prev_out"] is not None:
            zeros = (ctx["prev_out"],)
        else:
            zeros = ctx["zero_fn"]()
        outs = ctx["sharded"](*operands, *zeros)
        shards = sorted(
            outs[0].addressable_shards,
            key=lambda s: s.index[0].start if s.index[0].start else 0,
        )
        for s in shards:
            s.data.copy_to_host_async()
        res = np.empty((B, T, D), np.float32)
        for i, s in enumerate(shards):
            res[i] = np.asarray(s.data).astype(np.float32)
    except Exception:
        ctx["prev_out"] = None
        raise
    ctx["prev_out"] = outs[0]
    return res


# revision 34
# speedup vs baseline: 2.8109x; 1.0094x over previous
# Trainium2 Bass kernel for masked (key-padding) attention layer.
#
#   q,k,v = x@Wq, x@Wk, x@Wv ; score = q@k^T/sqrt(T) masked over keys;
#   out = softmax(score)@v @ Wo
#
# Sharding: data-parallel over batch, B=8 -> one batch element per NeuronCore.
#
# This is the sparse_attention problem: the key-padding mask kills ~half the
# keys (mask ~ Bernoulli(0.5)), and masked keys contribute exactly nothing
# (exp(-inf) = 0).  The host therefore COMPACTS the keys per batch element
# (gathers the rows with mask=1, padded to K = KT*128 with zero rows whose
# bias is -30000), which cuts every key-dimension GEMM to ~K/T = ~9/16 of the
# dense cost.  The device is a pure GEMM pipeline:
#
#   u[x,j]  = sum_x' A[x,x'] xkT[x',j]   A = 64*Wq@Wk^T folded on host  (A1)
#   v2[j,o] = sum_x xkT[x,j] Avo[x,o]    Avo = 64*Wv@Wo folded on host  (A2)
#   sT[j,t] = sum_x u[x,j] xT[x,t]                                      (B)
#   eT      = exp(sT/(64 sqrt(T)) + kbias)   ScalarE, PSUM->SBUF bf16
#   den[t]  = 64 * sum_j eT[j,t]         PE matmul against a 64s vector
#   out[t,o]= (sum_j eT[j,t] v2[j,o]) / den[t]   (C + ScalarE scale, bf16)
#
# A1/A2/B run as RESIDUAL-FP8 DoubleRow matmuls: each operand is split into
# an fp8e4 value plus an fp8e4 residual (v = v8 + vr with |vr| <~ 6% |v|),
# and each product takes 3 of the 4 cross terms (v8*w8 + v8*wr + vr*w8,
# dropping the ~0.4% vr*wr term) accumulated in one PSUM group.  DoubleRow
# processes 2 rows/cycle, so 3 fp8 passes = 0.75x the bf16 cost with ~1e-3
# relative error.  The x/xk/A/Avo splits are precomputed on the host (free);
# u's split costs one ScalarE copy + one DVE subtract per PSUM tile.  The
# x64 pre-scale on A/Avo keeps their N(0, 1/512) entries out of the fp8e4
# subnormal range; it is folded back via the exp scale and the 64s vector.
#
# Everything is laid out by the HOST so the device does no transposes and no
# collectives; stage C stays bf16 (exp values would need fp8 splits that
# saturate the ScalarE).  The PE is the bottleneck engine (~65us of matmul);
# a short zeros-matmul warmup keeps it busy (and its p-state ramped) under
# the initial input DMA so it never idles: any PE gap >100ns re-triggers the
# 3us mid-speed ramp.
import math

import numpy as np
import ml_dtypes

B = 8
T = 2048
D = 512
P = 128
KC = D // P       # 4 contraction chunks of 128
KG = KC // 2      # 2 double-row groups
QB = 512          # free-dim chunk (one PSUM bank of f32)
NQ = T // QB      # 4 query chunks
NT = T // P       # 16 query tiles of 128
WSCALE = 64.0
SCALE = 1.0 / math.sqrt(float(T)) / WSCALE
PAD_BIAS = -30000.0
NWARM = 9         # zeros-matmul PE warmup instructions

_BF16 = ml_dtypes.bfloat16
_FP8 = ml_dtypes.float8_e4m3

# blob row offsets (512-wide fp8 rows); value/residual planes interleaved
# per partition so each logical tensor is ONE dma (HWDGE issue is ~630ns
# per dma, serialized, so fewer+larger transfers win).
ATB0 = 0                    # fp8 (64 A^T | resid)  [128,2,4,512] -> 1024 rows
AVB0 = 1024                 # fp8 (64 Avo | resid)  [128,2,4,512] -> 1024 rows
XTB0 = 2048                 # fp8 (xT | resid)      [128,2,4,2048] -> 4096 rows
RBLOB8 = 6144

_ctx: dict = {}


def _build(kt: int):
    """Build + compile the single-core SPMD program for KT=kt key tiles."""
    import concourse.bass as bass
    import concourse.mybir as mybir
    import concourse.tile as tile
    from concourse import bacc

    dt = mybir.dt
    f32, bf16, fp8 = dt.float32, dt.bfloat16, dt.float8e4
    K = kt * P

    nc = bacc.Bacc(
        "TRN2",
        target_bir_lowering=False,
        debug=False,
        enable_asserts=False,
        num_devices=B,
    )

    blob_d = nc.dram_tensor("blob", [RBLOB8, D], fp8, kind="ExternalInput")
    # compacted+transposed keys (value + residual), rows (p, c) so any
    # key-range slice is a clean per-row run: row p*4+c holds xk[:, c*128+p].
    xkb_d = nc.dram_tensor("xkb", [P * 2 * KC, K], fp8, kind="ExternalInput")
    kb_d = nc.dram_tensor("kb", [P, 16], f32, kind="ExternalInput")
    out_d = nc.dram_tensor("out", [T, D], bf16, kind="ExternalOutput")

    Exp = mybir.ActivationFunctionType.Exp
    DR = mybir.MatmulPerfMode.DoubleRow

    # key-chunk boundaries for A1 / the xk DMA split
    kchunks = []
    k0 = 0
    while k0 < K:
        kchunks.append((k0, min(k0 + QB, K)))
        k0 += QB

    with tile.TileContext(nc) as tc:
        with (
            tc.tile_pool(name="const", bufs=1) as cpool,
            tc.tile_pool(name="big", bufs=1) as bpool,
            tc.tile_pool(name="psum", bufs=8, space="PSUM") as psum,
            tc.tile_pool(name="outs", bufs=2) as opool,
            tc.tile_pool(name="small", bufs=4) as spool,
        ):
            # ---- persistent SBUF tensors ----
            xTB = bpool.tile([P, 2, KC, T], fp8, tag="xTB")
            xkB = bpool.tile([P, 2, KC, K], fp8, tag="xkB")
            ATB = cpool.tile([P, 2, KC, D], fp8, tag="ATB")
            AVB = cpool.tile([P, 2, KC, D], fp8, tag="AVB")
            kbias = cpool.tile([P, 16], f32, tag="kbias")
            uB = bpool.tile([P, 2, KC, K], fp8, tag="uB")
            v2 = bpool.tile([P, kt, D], bf16, tag="v2")
            eT = bpool.tile([P, kt, T], bf16, tag="eT")
            zeros = cpool.tile([P, QB], bf16, tag="zeros")
            ones = cpool.tile([P, 1], bf16, tag="ones")

            # ---- PE warmup feed + 64s vector for the denominator ----
            nc.vector.memset(zeros[:], 0.0)
            nc.vector.memset(ones[:], WSCALE)

            # ---- input DMAs, in critical-path order ----
            # the A-matrix and first key chunk arrive value-plane first:
            # mm3's pass order is (v,v),(v,r),(r,v), so the first matmuls
            # need only the value planes and start ~1.3us earlier.
            atb_src = blob_d.ap()[ATB0 : ATB0 + 2 * KC * P, :].rearrange(
                "(p v c) w -> p v c w", p=P, v=2
            )
            xk0_src = xkb_d.ap()[:, 0 : kchunks[0][1]].rearrange(
                "(p v c) k -> p v c k", p=P, v=2
            )
            nc.sync.dma_start(ATB[:, 0], atb_src[:, 0])
            nc.sync.dma_start(xkB[:, 0, :, 0 : kchunks[0][1]], xk0_src[:, 0])
            nc.sync.dma_start(xkB[:, 1, :, 0 : kchunks[0][1]], xk0_src[:, 1])
            nc.sync.dma_start(ATB[:, 1], atb_src[:, 1])
            for j0, j1 in kchunks[1:]:
                nc.sync.dma_start(
                    xkB[:, :, :, j0:j1],
                    xkb_d.ap()[:, j0:j1].rearrange(
                        "(p v c) k -> p v c k", p=P, v=2
                    ),
                )
            nc.sync.dma_start(
                AVB[:],
                blob_d.ap()[AVB0 : AVB0 + 2 * KC * P, :].rearrange(
                    "(p v c) w -> p v c w", p=P, v=2
                ),
            )
            nc.sync.dma_start(kbias[:], kb_d.ap())
            nc.sync.dma_start(
                xTB[:],
                blob_d.ap()[XTB0 : XTB0 + 4096, :].rearrange(
                    "(p v c r) w -> p v c (r w)", p=P, v=2, c=KC
                ),
            )

            # ---- PE warmup: ramp the p-state under the input DMAs ----
            wt = psum.tile([P, QB], f32, tag="ps", name="ps")
            for _ in range(NWARM):
                nc.tensor.matmul(
                    wt[:], zeros[:, 0:P], zeros[:], start=True, stop=True
                )

            def mm3(out, LT, RT, lsl, rsl,
                    passes=((0, 0), (0, 1), (1, 0))):
                """residual-fp8 product into one PSUM group; default takes
                the 3 cross terms value*value + value*resid + resid*value."""
                n = 0
                for lv, rv in passes:
                    for g in range(KG):
                        gs = slice(2 * g, 2 * g + 2)
                        nc.tensor.matmul(
                            out,
                            LT[:, lv, gs, lsl],
                            RT[:, rv, gs, rsl],
                            start=(n == 0),
                            stop=(n == len(passes) * KG - 1),
                            perf_mode=DR,
                        )
                        n += 1

            # ---- stage A1: u[x, j] = (64 A) @ xkT, then split u -> u8+ur ----
            for j0, j1 in kchunks:
                cw = j1 - j0
                for m in range(KC):
                    pk = psum.tile([P, QB], f32, tag="ps", name="ps")
                    mm3(pk[:, :cw], ATB, xkB,
                        slice(m * P, (m + 1) * P), slice(j0, j1))
                    nc.scalar.copy(uB[:, 0, m, j0:j1], pk[:, :cw])

            # ---- stage A2: v2[j, o] = xk @ (64 Avo), bf16 result ----
            for j in range(kt):
                pv = psum.tile([P, D], f32, tag="ps", name="ps")
                mm3(pv[:], xkB, AVB,
                    slice(j * P, (j + 1) * P), slice(0, D))
                nc.vector.tensor_copy(v2[:, j, :], pv[:])

            # ---- stages B (scores+exp) and C (output), pipelined per
            # 512-query chunk so C consumes eT while B fills the next chunk.
            def bstage(tq):
                sl = slice(tq * QB, (tq + 1) * QB)
                for j in range(kt):
                    ps = psum.tile([P, QB], f32, tag="ps", name="ps")
                    mm3(ps[:], uB, xTB,
                        slice(j * P, (j + 1) * P), sl,
                        passes=((0, 0), (0, 1)))
                    nc.scalar.activation(
                        eT[:, j, sl],
                        ps[:],
                        Exp,
                        bias=kbias[:, j : j + 1],
                        scale=SCALE,
                    )

            def cstage(tq):
                for q in range(NQ):
                    tt = tq * NQ + q
                    tsl = slice(tt * P, (tt + 1) * P)
                    # denominator first: the DVE reciprocal overlaps the
                    # C-tile matmuls that follow.
                    dps = psum.tile([P, 1], f32, tag="ps", name="ps")
                    for j in range(kt):
                        nc.tensor.matmul(
                            dps[:],
                            eT[:, j, tsl],
                            ones[:],
                            start=(j == 0),
                            stop=(j == kt - 1),
                        )
                    rp = spool.tile([P, 1], f32, tag="rp", name="rp")
                    nc.vector.reciprocal(rp[:], dps[:])
                    po = psum.tile([P, D], f32, tag="ps", name="ps")
                    for j in range(kt):
                        nc.tensor.matmul(
                            po[:],
                            eT[:, j, tsl],
                            v2[:, j, :],
                            start=(j == 0),
                            stop=(j == kt - 1),
                        )
                    osb = opool.tile([P, QB], bf16, tag="osb", name="osb")
                    nc.scalar.mul(osb[:], po[:], rp[:])
                    nc.sync.dma_start(
                        out_d.ap()[tt * P : (tt + 1) * P, :], osb[:]
                    )

            # B runs one chunk ahead of C so the exp chain of chunk tq
            # finishes under B(tq+1)'s matmuls and C never waits.
            bstage(0)
            for tq in range(1, NQ):
                bstage(tq)
                cstage(tq - 1)
            cstage(NQ - 1)

    nc.compile()
    return nc


def _get_ctx(kt: int):
    """Build the program and a cached jitted executable (once per KT)."""
    if kt in _ctx:
        return _ctx[kt]
    import jax
    import jax.numpy as jnp
    from jax.experimental.shard_map import shard_map
    from jax.sharding import Mesh, PartitionSpec, NamedSharding
    import concourse.mybir as mybir
    from concourse import bass2jax

    bass2jax.install_neuronx_cc_hook()
    nc = _build(kt)
    partition_name = nc.partition_id_tensor.name if nc.partition_id_tensor else None
    in_names, out_names, out_avals = [], [], []
    for alloc in nc.m.functions[0].allocations:
        if not isinstance(alloc, mybir.MemoryLocationSet):
            continue
        name = alloc.memorylocations[0].name
        if alloc.kind == "ExternalInput":
            if name != partition_name:
                in_names.append(name)
        elif alloc.kind == "ExternalOutput":
            out_names.append(name)
            shape = tuple(alloc.tensor_shape)
            dtype = mybir.dt.np(alloc.dtype)
            out_avals.append(jax.core.ShapedArray(shape, dtype))
    n_params = len(in_names)
    n_outs = len(out_avals)
    all_names = list(in_names) + out_names
    if partition_name is not None:
        all_names = all_names + [partition_name]
    donate = tuple(range(n_params, n_params + n_outs))

    def _body(*args):
        operands = list(args)
        if partition_name is not None:
            operands.append(bass2jax.partition_id_tensor())
        outs = bass2jax._bass_exec_p.bind(
            *operands,
            out_avals=tuple(out_avals),
            in_names=tuple(all_names),
            out_names=tuple(out_names),
            lowering_input_output_aliases=(),
            sim_require_finite=True,
            sim_require_nnan=True,
            nc=nc,
        )
        return tuple(outs)

    devices = jax.devices()[:B]
    mesh = Mesh(np.asarray(devices), ("core",))
    in_specs = (PartitionSpec("core"),) * (n_params + n_outs)
    out_specs = (PartitionSpec("core"),) * n_outs
    sharded = jax.jit(
        shard_map(_body, mesh=mesh, in_specs=in_specs, out_specs=out_specs,
                  check_rep=False),
        donate_argnums=donate,
        keep_unused=True,
    )

    csh = NamedSharding(mesh, PartitionSpec("core"))
    zero_fn = jax.jit(
        lambda: tuple(
            jnp.zeros((B * a.shape[0],) + tuple(a.shape[1:]), a.dtype)
            for a in out_avals
        ),
        out_shardings=(csh,) * n_outs,
    )

    ctx = dict(
        nc=nc,
        in_names=in_names,
        sharded=sharded,
        zero_fn=zero_fn,
        prev_out=None,
    )
    _ctx[kt] = ctx
    return ctx


def _f8split(a):
    """fp8 value + fp8 residual of a float32 array."""
    v = a.astype(_FP8)
    r = (a - v.astype(np.float32)).astype(_FP8)
    return v, r


def _prep_args(x, mask, W_q, W_k, W_v, W_o, kt: int):
    """Host-side prep: fp8 value/residual splits in device SBUF layouts."""
    K = kt * P
    x = np.asarray(x, np.float32)
    mask = np.asarray(mask)
    blob = np.empty((B, RBLOB8, D), _FP8)
    xkb = np.zeros((B, P, 2, KC, K), _FP8)

    wq = np.asarray(W_q, np.float32)
    wk = np.asarray(W_k, np.float32)
    wv = np.asarray(W_v, np.float32)
    wo = np.asarray(W_o, np.float32)
    a = (wq @ wk.T) * WSCALE    # [x, x']; score = x @ A @ x^T / 64
    avo = (wv @ wo) * WSCALE    # [x, o];  64 * attn-value product
    at = a.T.reshape(KC, P, D).transpose(1, 0, 2)      # [128, 4, 512]
    av = avo.reshape(KC, P, D).transpose(1, 0, 2)
    atb = np.stack(_f8split(at), axis=1)               # [128, 2, 4, 512]
    avb = np.stack(_f8split(av), axis=1)
    blob[:, ATB0 : ATB0 + 2 * KC * P] = atb.reshape(2 * KC * P, D)
    blob[:, AVB0 : AVB0 + 2 * KC * P] = avb.reshape(2 * KC * P, D)

    # xT value/residual planes: [128, 2, 4, 2048] -> 32 rows per partition
    xt = np.ascontiguousarray(x.reshape(B, T, KC, P).transpose(0, 3, 2, 1))
    xtb = np.stack(_f8split(xt), axis=2)               # [B, 128, 2, 4, 2048]
    blob[:, XTB0 : XTB0 + 4096] = xtb.reshape(B, 4096, D)

    # per-core key compaction + bias
    kb = np.full((B, P, 16), PAD_BIAS, np.float32)
    for b in range(B):
        idx = np.flatnonzero(mask[b])
        kn = len(idx)
        xkt = x[b][idx].reshape(kn, KC, P).transpose(2, 1, 0)  # [128, 4, kn]
        v, r = _f8split(np.ascontiguousarray(xkt))
        xkb[b, :, 0, :, :kn] = v
        xkb[b, :, 1, :, :kn] = r
        jt, pp = np.divmod(np.arange(kn), P)
        kb[b, pp, jt] = 0.0
    return {
        "blob": blob.reshape(B * RBLOB8, D),
        "xkb": xkb.reshape(B * P * 2 * KC, K),
        "kb": kb.reshape(B * P, 16),
    }


def kernel(x, mask, W_q, W_k, W_v, W_o):
    mask = np.asarray(mask)
    counts = (mask != 0).sum(axis=1)
    kt = max(1, int(-(-int(counts.max()) // P)))
    ctx = _get_ctx(kt)
    args = _prep_args(x, mask, W_q, W_k, W_v, W_o, kt)
    operands = [args[name] for name in ctx["in_names"]]
    try:
        if ctx["prev_out"] is not None:
            zeros = (ctx["prev_out"],)
        else:
            zeros = ctx["zero_fn"]()
        outs = ctx["sharded"](*operands, *zeros)
        shards = sorted(
            outs[0].addressable_shards,
            key=lambda s: s.index[0].start if s.index[0].start else 0,
        )
        for s in shards:
            s.data.copy_to_host_async()
        res = np.empty((B, T, D), np.float32)
        for i, s in enumerate(shards):
            res[i] = np.asarray(s.data).astype(np.float32)
    except Exception:
        ctx["prev_out"] = None
        raise
    ctx["prev_out"] = outs[0]
    return res


# revision 38
# speedup vs baseline: 2.9338x; 1.0437x over previous
# Trainium2 Bass kernel for masked (key-padding) attention layer.
#
#   q,k,v = x@Wq, x@Wk, x@Wv ; score = q@k^T/sqrt(T) masked over keys;
#   out = softmax(score)@v @ Wo
#
# Sharding: data-parallel over batch, B=8 -> one batch element per NeuronCore.
#
# This is the sparse_attention problem: the key-padding mask kills ~half the
# keys (mask ~ Bernoulli(0.5)), and masked keys contribute exactly nothing
# (exp(-inf) = 0).  The host therefore COMPACTS the keys per batch element
# (gathers the rows with mask=1, padded to K = KT*128 with zero rows whose
# bias is -30000), which cuts every key-dimension GEMM to ~K/T = ~9/16 of the
# dense cost.  The device is a pure GEMM pipeline:
#
#   u[x,j]  = sum_x' A[x,x'] xkT[x',j]   A = 64*Wq@Wk^T folded on host  (A1)
#   v2[j,o] = sum_x xkT[x,j] Avo[x,o]    Avo = 64*Wv@Wo folded on host  (A2)
#   sT[j,t] = sum_x u[x,j] xT[x,t]                                      (B)
#   eT      = exp(sT/(64 sqrt(T)) + kbias)   ScalarE, PSUM->SBUF bf16
#   den[t]  = 64 * sum_j eT[j,t]         PE matmul against a 64s vector
#   out[t,o]= (sum_j eT[j,t] v2[j,o]) / den[t]   (C + ScalarE scale, bf16)
#
# A1/A2/B run as RESIDUAL-FP8 DoubleRow matmuls: each operand is split into
# an fp8e4 value plus an fp8e4 residual (v = v8 + vr with |vr| <~ 6% |v|),
# and each product takes 3 of the 4 cross terms (v8*w8 + v8*wr + vr*w8,
# dropping the ~0.4% vr*wr term) accumulated in one PSUM group.  DoubleRow
# processes 2 rows/cycle, so 3 fp8 passes = 0.75x the bf16 cost with ~1e-3
# relative error.  The x/xk/A/Avo splits are precomputed on the host (free);
# u's split costs one ScalarE copy + one DVE subtract per PSUM tile.  The
# x64 pre-scale on A/Avo keeps their N(0, 1/512) entries out of the fp8e4
# subnormal range; it is folded back via the exp scale and the 64s vector.
#
# Everything is laid out by the HOST so the device does no transposes and no
# collectives; stage C stays bf16 (exp values would need fp8 splits that
# saturate the ScalarE).  The PE is the bottleneck engine (~65us of matmul);
# a short zeros-matmul warmup keeps it busy (and its p-state ramped) under
# the initial input DMA so it never idles: any PE gap >100ns re-triggers the
# 3us mid-speed ramp.
import math

import numpy as np
import ml_dtypes

B = 8
T = 2048
D = 512
P = 128
KC = D // P       # 4 contraction chunks of 128
KG = KC // 2      # 2 double-row groups
QB = 512          # free-dim chunk (one PSUM bank of f32)
NQ = T // QB      # 4 query chunks
NT = T // P       # 16 query tiles of 128
WSCALE = 64.0
SCALE = 1.0 / math.sqrt(float(T)) / WSCALE
PAD_BIAS = -30000.0
NWARM = 9         # zeros-matmul PE warmup instructions

_BF16 = ml_dtypes.bfloat16
_FP8 = ml_dtypes.float8_e4m3

# blob row offsets (512-wide fp8 rows); value/residual planes interleaved
# per partition so each logical tensor is ONE dma (HWDGE issue is ~630ns
# per dma, serialized, so fewer+larger transfers win).
ATB0 = 0                    # fp8 (64 A^T | resid)  [128,2,4,512] -> 1024 rows
AVB0 = 1024                 # fp8 (64 Avo | resid)  [128,2,4,512] -> 1024 rows
XTB0 = 2048                 # fp8 xT (value only)   [128,4,2048] -> 2048 rows
RBLOB8 = 4096

_ctx: dict = {}


def _build(kt: int):
    """Build + compile the single-core SPMD program for KT=kt key tiles."""
    import concourse.bass as bass
    import concourse.mybir as mybir
    import concourse.tile as tile
    from concourse import bacc

    dt = mybir.dt
    f32, bf16, fp8 = dt.float32, dt.bfloat16, dt.float8e4
    K = kt * P

    nc = bacc.Bacc(
        "TRN2",
        target_bir_lowering=False,
        debug=False,
        enable_asserts=False,
        num_devices=B,
    )

    blob_d = nc.dram_tensor("blob", [RBLOB8, D], fp8, kind="ExternalInput")
    # compacted+transposed keys (value + residual), rows (p, c) so any
    # key-range slice is a clean per-row run: row p*4+c holds xk[:, c*128+p].
    xkb_d = nc.dram_tensor("xkb", [P * 2 * KC, K], fp8, kind="ExternalInput")
    kb_d = nc.dram_tensor("kb", [P, 16], f32, kind="ExternalInput")
    out_d = nc.dram_tensor("out", [T, D], bf16, kind="ExternalOutput")

    Exp = mybir.ActivationFunctionType.Exp
    DR = mybir.MatmulPerfMode.DoubleRow

    # key-chunk boundaries for A1 / the xk DMA split
    kchunks = []
    k0 = 0
    while k0 < K:
        kchunks.append((k0, min(k0 + QB, K)))
        k0 += QB

    with tile.TileContext(nc) as tc:
        with (
            tc.tile_pool(name="const", bufs=1) as cpool,
            tc.tile_pool(name="big", bufs=1) as bpool,
            tc.tile_pool(name="psum", bufs=8, space="PSUM") as psum,
            tc.tile_pool(name="outs", bufs=2) as opool,
            tc.tile_pool(name="small", bufs=4) as spool,
        ):
            # ---- persistent SBUF tensors ----
            xTB = bpool.tile([P, 1, KC, T], fp8, tag="xTB")
            xkB = bpool.tile([P, 2, KC, K], fp8, tag="xkB")
            ATB = cpool.tile([P, 2, KC, D], fp8, tag="ATB")
            AVB = cpool.tile([P, 2, KC, D], fp8, tag="AVB")
            kbias = cpool.tile([P, 16], f32, tag="kbias")
            uB = bpool.tile([P, 2, KC, K], fp8, tag="uB")
            v2 = bpool.tile([P, kt, D], bf16, tag="v2")
            eT = bpool.tile([P, kt, T], bf16, tag="eT")
            zeros = cpool.tile([P, QB], bf16, tag="zeros")
            ones = cpool.tile([P, 1], bf16, tag="ones")

            # ---- PE warmup feed + 64s vector for the denominator ----
            nc.vector.memset(zeros[:], 0.0)
            nc.vector.memset(ones[:], WSCALE)

            # ---- input DMAs, in critical-path order ----
            # the A-matrix and first key chunk arrive value-plane first:
            # mm3's pass order is (v,v),(v,r),(r,v), so the first matmuls
            # need only the value planes and start ~1.3us earlier.
            atb_src = blob_d.ap()[ATB0 : ATB0 + 2 * KC * P, :].rearrange(
                "(p v c) w -> p v c w", p=P, v=2
            )
            xk0_src = xkb_d.ap()[:, 0 : kchunks[0][1]].rearrange(
                "(p v c) k -> p v c k", p=P, v=2
            )
            nc.sync.dma_start(ATB[:, 0], atb_src[:, 0])
            nc.sync.dma_start(xkB[:, 0, :, 0 : kchunks[0][1]], xk0_src[:, 0])
            nc.sync.dma_start(xkB[:, 1, :, 0 : kchunks[0][1]], xk0_src[:, 1])
            nc.sync.dma_start(ATB[:, 1], atb_src[:, 1])
            for j0, j1 in kchunks[1:]:
                nc.sync.dma_start(
                    xkB[:, :, :, j0:j1],
                    xkb_d.ap()[:, j0:j1].rearrange(
                        "(p v c) k -> p v c k", p=P, v=2
                    ),
                )
            nc.sync.dma_start(
                AVB[:],
                blob_d.ap()[AVB0 : AVB0 + 2 * KC * P, :].rearrange(
                    "(p v c) w -> p v c w", p=P, v=2
                ),
            )
            nc.sync.dma_start(kbias[:], kb_d.ap())
            nc.sync.dma_start(
                xTB[:, 0],
                blob_d.ap()[XTB0 : XTB0 + 2048, :].rearrange(
                    "(p c r) w -> p c (r w)", p=P, c=KC
                ),
            )

            # ---- PE warmup: ramp the p-state under the input DMAs ----
            wt = psum.tile([P, QB], f32, tag="ps", name="ps")
            for _ in range(NWARM):
                nc.tensor.matmul(
                    wt[:], zeros[:, 0:P], zeros[:], start=True, stop=True
                )

            def mm3(out, LT, RT, lsl, rsl,
                    passes=((0, 0), (0, 1), (1, 0))):
                """residual-fp8 product into one PSUM group; default takes
                the 3 cross terms value*value + value*resid + resid*value."""
                n = 0
                for lv, rv in passes:
                    for g in range(KG):
                        gs = slice(2 * g, 2 * g + 2)
                        nc.tensor.matmul(
                            out,
                            LT[:, lv, gs, lsl],
                            RT[:, rv, gs, rsl],
                            start=(n == 0),
                            stop=(n == len(passes) * KG - 1),
                            perf_mode=DR,
                        )
                        n += 1

            # ---- stage A1: u[x, j] = (64 A) @ xkT, then split u -> u8+ur ----
            for j0, j1 in kchunks:
                cw = j1 - j0
                for m in range(KC):
                    pk = psum.tile([P, QB], f32, tag="ps", name="ps")
                    mm3(pk[:, :cw], ATB, xkB,
                        slice(m * P, (m + 1) * P), slice(j0, j1))
                    nc.scalar.copy(uB[:, 0, m, j0:j1], pk[:, :cw])

            # ---- stage A2: v2[j, o] = xk @ (64 Avo), bf16 result ----
            for j in range(kt):
                pv = psum.tile([P, D], f32, tag="ps", name="ps")
                mm3(pv[:], xkB, AVB,
                    slice(j * P, (j + 1) * P), slice(0, D))
                nc.vector.tensor_copy(v2[:, j, :], pv[:])

            # ---- stages B (scores+exp) and C (output), pipelined per
            # 512-query chunk so C consumes eT while B fills the next chunk.
            def bstage(tq):
                sl = slice(tq * QB, (tq + 1) * QB)
                for j in range(kt):
                    ps = psum.tile([P, QB], f32, tag="ps", name="ps")
                    mm3(ps[:], uB, xTB,
                        slice(j * P, (j + 1) * P), sl,
                        passes=((0, 0),))
                    nc.scalar.activation(
                        eT[:, j, sl],
                        ps[:],
                        Exp,
                        bias=kbias[:, j : j + 1],
                        scale=SCALE,
                    )

            def cstage(tq):
                for q in range(NQ):
                    tt = tq * NQ + q
                    tsl = slice(tt * P, (tt + 1) * P)
                    # denominator first: the DVE reciprocal overlaps the
                    # C-tile matmuls that follow.
                    dps = psum.tile([P, 1], f32, tag="ps", name="ps")
                    for j in range(kt):
                        nc.tensor.matmul(
                            dps[:],
                            eT[:, j, tsl],
                            ones[:],
                            start=(j == 0),
                            stop=(j == kt - 1),
                        )
                    rp = spool.tile([P, 1], f32, tag="rp", name="rp")
                    nc.vector.reciprocal(rp[:], dps[:])
                    po = psum.tile([P, D], f32, tag="ps", name="ps")
                    for j in range(kt):
                        nc.tensor.matmul(
                            po[:],
                            eT[:, j, tsl],
                            v2[:, j, :],
                            start=(j == 0),
                            stop=(j == kt - 1),
                        )
                    osb = opool.tile([P, QB], bf16, tag="osb", name="osb")
                    nc.scalar.mul(osb[:], po[:], rp[:])
                    nc.sync.dma_start(
                        out_d.ap()[tt * P : (tt + 1) * P, :], osb[:]
                    )

            # B runs one chunk ahead of C so the exp chain of chunk tq
            # finishes under B(tq+1)'s matmuls and C never waits.
            bstage(0)
            for tq in range(1, NQ):
                bstage(tq)
                cstage(tq - 1)
            cstage(NQ - 1)

    nc.compile()
    return nc


def _get_ctx(kt: int):
    """Build the program and a cached jitted executable (once per KT)."""
    if kt in _ctx:
        return _ctx[kt]
    import jax
    import jax.numpy as jnp
    from jax.experimental.shard_map import shard_map
    from jax.sharding import Mesh, PartitionSpec, NamedSharding
    import concourse.mybir as mybir
    from concourse import bass2jax

    bass2jax.install_neuronx_cc_hook()
    nc = _build(kt)
    partition_name = nc.partition_id_tensor.name if nc.partition_id_tensor else None
    in_names, out_names, out_avals = [], [], []
    for alloc in nc.m.functions[0].allocations:
        if not isinstance(alloc, mybir.MemoryLocationSet):
            continue
        name = alloc.memorylocations[0].name
        if alloc.kind == "ExternalInput":
            if name != partition_name:
                in_names.append(name)
        elif alloc.kind == "ExternalOutput":
            out_names.append(name)
            shape = tuple(alloc.tensor_shape)
            dtype = mybir.dt.np(alloc.dtype)
            out_avals.append(jax.core.ShapedArray(shape, dtype))
    n_params = len(in_names)
    n_outs = len(out_avals)
    all_names = list(in_names) + out_names
    if partition_name is not None:
        all_names = all_names + [partition_name]
    donate = tuple(range(n_params, n_params + n_outs))

    def _body(*args):
        operands = list(args)
        if partition_name is not None:
            operands.append(bass2jax.partition_id_tensor())
        outs = bass2jax._bass_exec_p.bind(
            *operands,
            out_avals=tuple(out_avals),
            in_names=tuple(all_names),
            out_names=tuple(out_names),
            lowering_input_output_aliases=(),
            sim_require_finite=True,
            sim_require_nnan=True,
            nc=nc,
        )
        return tuple(outs)

    devices = jax.devices()[:B]
    mesh = Mesh(np.asarray(devices), ("core",))
    in_specs = (PartitionSpec("core"),) * (n_params + n_outs)
    out_specs = (PartitionSpec("core"),) * n_outs
    sharded = jax.jit(
        shard_map(_body, mesh=mesh, in_specs=in_specs, out_specs=out_specs,
                  check_rep=False),
        donate_argnums=donate,
        keep_unused=True,
    )

    csh = NamedSharding(mesh, PartitionSpec("core"))
    zero_fn = jax.jit(
        lambda: tuple(
            jnp.zeros((B * a.shape[0],) + tuple(a.shape[1:]), a.dtype)
            for a in out_avals
        ),
        out_shardings=(csh,) * n_outs,
    )

    ctx = dict(
        nc=nc,
        in_names=in_names,
        sharded=sharded,
        zero_fn=zero_fn,
        prev_out=None,
    )
    _ctx[kt] = ctx
    return ctx


def _f8split(a):
    """fp8 value + fp8 residual of a float32 array."""
    v = a.astype(_FP8)
    r = (a - v.astype(np.float32)).astype(_FP8)
    return v, r


def _prep_args(x, mask, W_q, W_k, W_v, W_o, kt: int):
    """Host-side prep: fp8 value/residual splits in device SBUF layouts."""
    K = kt * P
    x = np.asarray(x, np.float32)
    mask = np.asarray(mask)
    blob = np.empty((B, RBLOB8, D), _FP8)
    xkb = np.zeros((B, P, 2, KC, K), _FP8)

    wq = np.asarray(W_q, np.float32)
    wk = np.asarray(W_k, np.float32)
    wv = np.asarray(W_v, np.float32)
    wo = np.asarray(W_o, np.float32)
    a = (wq @ wk.T) * WSCALE    # [x, x']; score = x @ A @ x^T / 64
    avo = (wv @ wo) * WSCALE    # [x, o];  64 * attn-value product
    at = a.T.reshape(KC, P, D).transpose(1, 0, 2)      # [128, 4, 512]
    av = avo.reshape(KC, P, D).transpose(1, 0, 2)
    atb = np.stack(_f8split(at), axis=1)               # [128, 2, 4, 512]
    avb = np.stack(_f8split(av), axis=1)
    blob[:, ATB0 : ATB0 + 2 * KC * P] = atb.reshape(2 * KC * P, D)
    blob[:, AVB0 : AVB0 + 2 * KC * P] = avb.reshape(2 * KC * P, D)

    # xT value plane: [128, 4, 2048] -> 16 rows per partition (stage B's
    # single fp8 pass reads only the value plane; u carries the residual
    # budget, see mm3's pass list)
    xt = np.ascontiguousarray(x.reshape(B, T, KC, P).transpose(0, 3, 2, 1))
    blob[:, XTB0 : XTB0 + 2048] = xt.astype(_FP8).reshape(B, 2048, D)

    # per-core key compaction + bias
    kb = np.full((B, P, 16), PAD_BIAS, np.float32)
    for b in range(B):
        idx = np.flatnonzero(mask[b])
        kn = len(idx)
        xkt = x[b][idx].reshape(kn, KC, P).transpose(2, 1, 0)  # [128, 4, kn]
        v, r = _f8split(np.ascontiguousarray(xkt))
        xkb[b, :, 0, :, :kn] = v
        xkb[b, :, 1, :, :kn] = r
        jt, pp = np.divmod(np.arange(kn), P)
        kb[b, pp, jt] = 0.0
    return {
        "blob": blob.reshape(B * RBLOB8, D),
        "xkb": xkb.reshape(B * P * 2 * KC, K),
        "kb": kb.reshape(B * P, 16),
    }


def kernel(x, mask, W_q, W_k, W_v, W_o):
    mask = np.asarray(mask)
    counts = (mask != 0).sum(axis=1)
    kt = max(1, int(-(-int(counts.max()) // P)))
    ctx = _get_ctx(kt)
    args = _prep_args(x, mask, W_q, W_k, W_v, W_o, kt)
    operands = [args[name] for name in ctx["in_names"]]
    try:
        if ctx["prev_out"] is not None:
            zeros = (ctx["prev_out"],)
        else:
            zeros = ctx["zero_fn"]()
        outs = ctx["sharded"](*operands, *zeros)
        shards = sorted(
            outs[0].addressable_shards,
            key=lambda s: s.index[0].start if s.index[0].start else 0,
        )
        for s in shards:
            s.data.copy_to_host_async()
        res = np.empty((B, T, D), np.float32)
        for i, s in enumerate(shards):
            res[i] = np.asarray(s.data).astype(np.float32)
    except Exception:
        ctx["prev_out"] = None
        raise
    ctx["prev_out"] = outs[0]
    return res
